# revision 11
# baseline (speedup 1.0000x reference)
"""Ewald summation kernel for Trainium2 (8 NeuronCores, SPMD).

Strategy
--------
Host (numpy, O(B*K + N) work):
  * Build the 21^3 integer reciprocal lattice, mirror the reference's fp32
    weight computation, and keep only k-points with nonzero weight
    (~460 of 9261 per batch -- exact: zero-weight points contribute 0).
  * Shard by batch: B=16 batches over 8 cores, 2 per core.  Each core
    computes complete structure factors for its two batches, so no
    collective is needed; per-core output is 8 partial scalars.
  * Work in "turns": phase/2pi = nvec . f where f = inv(cell) r are
    fractional coordinates.  f is wrapped to [-1/2, 1/2) and split into
    a 12-bit head + tail (both exact in float32r, as are the integer
    nvec), so ONE single-pass float32r matmul with contraction 6 gives
    the phase to fp32 accuracy.  The wrap shifts phases by multiples of
    1/2 turn per k-point, which only flips the sign of (sin, cos)
    together -- invariant under |S|^2.

Device (per core, Bass/Tile), per 128-atom chunk (NCH chunks, uniform):
  PE  : ph[128, KP] = f6^T @ nv6            (float32r, one matmul)
  DVE : d_sin = ph - round(ph)              (custom ROUND_REDUCE op)
        d_cos = wrap(d_sin + 1/4)           (ADD_RANGE_WRAP custom op)
  ACT : trig[128, 2KP] = Sin(2pi * [d_sin | d_cos])   -> float32r
  PE  : S[8, KP] += q8^T @ trig_half        (float32r matmuls; psum rows
        are (batch_local, half, channel))
tail: t2 = (S*w8)*S, reduce_sum -> [8,1] -> DMA out.

Host combines: pot[b] = sum(rows of b)/vol[b] - 2*self_term[b], * NORM.
"""

import os
import numpy as np

import concourse.bass as bass
import concourse.tile as tile
from concourse import bacc, mybir
from concourse.bass_utils import run_bass_kernel_spmd

# --- problem constants (from the reference model) -------------------------
N_MAX = 10
DL = 2.0
SIGMA = 1.0
NORM_FACTOR = 90.0474
TWOPI = 2.0 * np.pi
K_SQ_MAX = (TWOPI / DL) ** 2
SIGMA_SQ_HALF = SIGMA ** 2 / 2.0

N_CORES = 8
MAGIC = float(1.5 * 2 ** 23)  # fp32 round-to-nearest-integer magic constant

_last_results = None  # BassKernelResults of the most recent run (for test.py)


def _register_round_ops():
    """Custom DVE ops:
    ROUND_REDUCE_ANT:       out = in0 - ((in0 + s0) - s0) = in0 - round(in0)
    SHIFT_ROUND_REDUCE_ANT: out = y - ((y + s0) - s0), y = in0 + s1
    (s0 = fp32 magic rounding constant)."""
    import concourse.dve_ops as dve_ops
    from concourse.dve_spec import Spec, Src0, C0, C1, C2, PageIdx, lower
    from concourse.dve_uop import DveOpSpec

    def reg(name, spec, subdim=False):
        for op in dve_ops.OPS:
            if op.name == name:
                return op
        row = dve_ops._CUSTOM_DVE_ROW_BASE + len(dve_ops.OPS)
        assert row < 0x20
        dve_ops._SUB_OPCODE_FOR_NAME[name] = row
        shas = {}
        for ver in ("v3", "v4"):
            sp = DveOpSpec(name=name, opcode=row, uops=lower(spec, ver=ver),
                           rd1_en=False)
            shas[ver] = sp.sha(ver)
        op = dve_ops.DveOp(name, spec, subdim=subdim, uops_sha=shas)
        dve_ops.OPS.append(op)
        dve_ops.CUSTOM_DVE_SPECS[name] = spec
        return op

    r1 = reg("ROUND_REDUCE_ANT", Spec(
        body=Src0 - ((Src0 + C0) - C0),
        reference=lambda in0, in1, s0, s1, imm2: (
            in0 - ((in0 + s0) - s0)).astype(np.float32)))
    r2 = reg("SHIFT_ROUND_REDUCE_ANT", Spec(
        body=(Src0 + C1) - (((Src0 + C1) + C0) - C0),
        reference=lambda in0, in1, s0, s1, imm2: (
            (in0 + s1) - (((in0 + s1) + s0) - s0)).astype(np.float32)))

    def _pref(in0, in1, s0, s1, imm2):
        out = np.empty_like(in0)
        for pg in range(in0.shape[1]):
            y = in0[:, pg, :] + (s0 + s1 * pg)
            out[:, pg, :] = y - ((y + imm2) - imm2)
        return out.astype(np.float32)

    _y = Src0 + PageIdx(C0, C1)
    r3 = reg3 = None
    r3 = reg("PAGED_ROUND_REDUCE_ANT", Spec(
        body=_y - ((_y + C2) - C2),
        reference=_pref), subdim=True)
    return r1, r2, r3


def _k_lattice():
    g = np.arange(-N_MAX, N_MAX + 1)
    nvec = np.stack(np.meshgrid(g, g, g, indexing="ij"), axis=-1).reshape(-1, 3)
    nonzero = nvec != 0
    has_nz = nonzero.any(axis=1)
    first_nz = np.argmax(nonzero.astype(np.int32), axis=1)
    sign = nvec[np.arange(nvec.shape[0]), first_nz]
    hemi = (sign > 0) | ~has_nz
    factors = np.where(~has_nz, 1.0, 2.0).astype(np.float32)
    return nvec, hemi, factors


def _host_prep(q, r, cell, batch):
    """All O(B*K + N) prep.  Returns per-core input maps + combine info."""
    q = np.asarray(q, np.float32)
    r = np.asarray(r, np.float32)
    cell = np.asarray(cell, np.float32)
    batch = np.asarray(batch)
    B = cell.shape[0]
    assert B == 2 * N_CORES, f"batch-pair sharding assumes B=16, got {B}"

    nvec, hemi, factors = _k_lattice()

    # fp32 weight computation mirroring the reference
    inv32 = np.linalg.inv(cell).astype(np.float32)          # [B,3,3]
    G = (TWOPI * np.transpose(inv32, (0, 2, 1))).astype(np.float32)
    kvec = np.einsum("kj,bji->bki", nvec.astype(np.float32), G).astype(np.float32)
    k_sq = (kvec ** 2).sum(-1)
    valid = (k_sq > 0) & (k_sq <= np.float32(K_SQ_MAX)) & hemi[None, :]
    w = (np.exp(-np.float32(SIGMA_SQ_HALF) * k_sq) / (k_sq + 1e-12)
         * factors[None, :] * valid)

    inv64 = np.linalg.inv(cell.astype(np.float64))          # [B,3,3]

    sel_idx = [np.nonzero(w[b])[0] for b in range(B)]
    kmax = max(len(i) for i in sel_idx)
    KP = min(512, ((kmax + 15) // 16) * 16)
    assert kmax <= 512, f"valid k-points {kmax} > 512 unsupported"

    nsel = np.zeros((B, KP, 3), np.float32)                 # integer nvec
    wsel = np.zeros((B, KP), np.float32)
    for b in range(B):
        idx = sel_idx[b]
        nsel[b, : len(idx)] = nvec[idx]
        wsel[b, : len(idx)] = w[b][idx]

    counts = np.bincount(batch, minlength=B)
    starts = np.concatenate([[0], np.cumsum(counts)[:-1]])
    CB = int(max((counts + 127) // 128))
    NCH = 2 * CB

    # fractional coords wrapped to [-1/2, 1/2), split 12-bit head + tail
    f_all = np.einsum("bji,nj->nbi", inv64, r.astype(np.float64))  # [N,B,3]
    vol = np.linalg.det(cell.astype(np.float64))
    q_sq = q.astype(np.float64) ** 2
    self_term = np.array(
        [q_sq[batch == b].sum() for b in range(B)]) / (SIGMA * TWOPI ** 1.5)

    in_maps = []
    for m in range(N_CORES):
        f6 = np.zeros((6, NCH * 128), np.float32)
        q8 = np.zeros((128, NCH * 2 * 8), np.float32)
        nv6 = np.zeros((6, 2 * KP), np.float32)
        w8 = np.zeros((8, KP), np.float32)
        for bl in range(2):
            gb = 2 * m + bl
            nv6[0:3, bl * KP:(bl + 1) * KP] = nsel[gb].T
            nv6[3:6, bl * KP:(bl + 1) * KP] = nsel[gb].T
            for h in range(2):
                for c in range(2):
                    w8[4 * bl + 2 * h + c] = wsel[gb]
            nb = int(counts[gb])
            s0 = int(starts[gb])
            for j in range(CB):
                ch = bl * CB + j
                lo = j * 128
                n_here = max(0, min(128, nb - lo))
                if n_here == 0:
                    continue
                rows = slice(s0 + lo, s0 + lo + n_here)
                fb = f_all[rows, gb, :]                     # [n,3] fp64
                fb = np.mod(fb, 1.0) - 0.5
                fh = np.trunc(fb * 4096.0) / 4096.0
                fl = (fb - fh).astype(np.float32)
                f6[0:3, ch * 128: ch * 128 + n_here] = fh.T.astype(np.float32)
                f6[3:6, ch * 128: ch * 128 + n_here] = fl.T
                for h in range(2):
                    base = (2 * ch + h) * 8
                    for c in range(2):
                        q8[:n_here, base + 4 * bl + 2 * h + c] = q[rows, c]
        fnv = np.concatenate([f6[:, 0:128], nv6[:, 0:KP],
                              f6[:, 128:], nv6[:, KP:]], axis=1)
        q8z = np.concatenate([q8, np.zeros((128, 1), np.float32)], axis=1)
        in_maps.append({"fnv": fnv, "q8z": q8z,
                        "w8": np.sqrt(w8).astype(np.float32)})

    meta = dict(KP=KP, CB=CB, NCH=NCH, vol=vol, self_term=self_term)
    return in_maps, meta


def _build_kernel(KP, NCH, CB):
    rop, rop2, rop3 = _register_round_ops()

    orig_barrier = bass.Bass.all_engine_barrier
    orig_memset = bass.BassGpSimd.memset
    bass.Bass.all_engine_barrier = lambda self, **kw: None
    bass.BassGpSimd.memset = lambda self, ap, constant: None
    try:
        nc = bacc.Bacc("TRN2", target_bir_lowering=False, debug=False,
                       num_devices=N_CORES, detect_race_conditions=False,
                       enable_partition_id=False, monotonic_sem_count=0)
    finally:
        bass.Bass.all_engine_barrier = orig_barrier
        bass.BassGpSimd.memset = orig_memset

    f32 = mybir.dt.float32
    f32r = mybir.dt.float32r
    fnv = nc.dram_tensor("fnv", [6, NCH * 128 + 2 * KP], f32r,
                         kind="ExternalInput")
    q8z = nc.dram_tensor("q8z", [128, NCH * 2 * 8 + 1], f32r,
                         kind="ExternalInput")
    w8 = nc.dram_tensor("w8", [8, KP], f32, kind="ExternalInput")
    out = nc.dram_tensor("out", [8, 1], f32, kind="ExternalOutput")

    # slim exit: drain + one sem-only barrier + sem clear (the NEFF-level
    # postamble barrier provides the final sync for re-execution)
    def _slim_drain_and_barrier(self, tick_clock, wait_clock):
        from concourse.tile import ScopedClock
        drain_inst = self.nc.sync.drain()
        wait_clock.add_sem_waits(
            drain_inst.ins, ScopedClock({None: tick_clock.global_clock}))
        self.nc.all_engine_barrier(sem_only=True)
        popped = self.nc._tile_sem_poison_stack.pop()
        assert popped is self._sem_poison
        self.nc.clear_and_free_semaphores(
            list(self.sems.allocated().values()))

    Sin = mybir.ActivationFunctionType.Sin
    Alu = mybir.AluOpType

    orig_dab = tile.TileContext._drain_and_barrier
    tile.TileContext._drain_and_barrier = _slim_drain_and_barrier
    try:
        _build_body(nc, rop3, KP, NCH, CB, fnv, q8z, w8, out, Sin, Alu)
    finally:
        tile.TileContext._drain_and_barrier = orig_dab
    nc.compile()
    return nc


def _build_body(nc, rop3, KP, NCH, CB, fnv, q8z, w8, out, Sin, Alu):
    f32 = mybir.dt.float32
    f32r = mybir.dt.float32r
    with tile.TileContext(nc) as tc:
        with tc.tile_pool(name="consts", bufs=1) as consts, \
             tc.tile_pool(name="work", bufs=3) as work, \
             tc.tile_pool(name="fin", bufs=1) as fin, \
             tc.tile_pool(name="php", bufs=5, space="PSUM") as php, \
             tc.tile_pool(name="pss", bufs=1, space="PSUM") as pss:

            fnv_t = consts.tile([6, NCH * 128 + 2 * KP], f32r)
            # chunk-0 lhsT + batch-0 nvec first so the first matmul can start
            nc.sync.dma_start(out=fnv_t[:, 0:128 + KP],
                              in_=fnv.ap()[:, 0:128 + KP])
            nc.sync.dma_start(out=fnv_t[:, 128 + KP:],
                              in_=fnv.ap()[:, 128 + KP:])
            q8z_t = consts.tile([128, NCH * 2 * 8 + 1], f32r)
            w8_t = consts.tile([8, KP], f32)
            nc.sync.dma_start(out=q8z_t, in_=q8z.ap())
            nc.sync.dma_start(out=w8_t, in_=w8.ap())
            zz_t = q8z_t.bitcast(f32)[:, NCH * 2 * 8:]

            def f6_sl(c):
                if c == 0:
                    return fnv_t[:, 0:128]
                return fnv_t[:, KP + c * 128:KP + (c + 1) * 128]

            def nv6_sl(bl):
                if bl == 0:
                    return fnv_t[:, 128:128 + KP]
                return fnv_t[:, 128 + KP + (NCH - 1) * 128:
                             128 + KP + (NCH - 1) * 128 + KP]

            # PE warmup: dummy matmul with no input deps so the PE fetches
            # its instruction block while the first DMAs are in flight.
            warm = work.tile([1, 16], f32, tag="warm")
            nc.vector.memset(warm, 0.0)
            wps = pss.tile([16, 16], f32, tag="warmps")
            nc.tensor.matmul(wps, warm[0:1, :], warm[0:1, :],
                             start=True, stop=True)

            s_ps = pss.tile([8, KP], f32)

            for c in range(NCH):
                bl = c // CB
                ph = php.tile([128, KP], f32, tag="ph")
                nc.tensor.matmul(ph, f6_sl(c), nv6_sl(bl),
                                 start=True, stop=True)
                d2 = work.tile([128, 2, KP], f32, tag="d2")
                ph2 = bass.AP(tensor=ph.tensor, offset=ph.offset,
                              ap=[ph.ap[0], [0, 2], ph.ap[1]])
                # one paged op: page 0 -> sin d, page 1 -> +1/4 turn (cos)
                nc.vector._custom_dve(rop3, out=d2, in0=ph2,
                                      s0=0.0, s1=0.25, imm2=MAGIC)
                trig = work.tile([128, 2, KP], f32r, tag="trig")
                nc.scalar.activation(out=trig, in_=d2, func=Sin,
                                     bias=zz_t[:, 0:1], scale=float(TWOPI))
                for h in range(2):
                    nc.tensor.matmul(
                        s_ps[:, 0:KP],
                        q8z_t[:, (2 * c + h) * 8:(2 * c + h + 1) * 8],
                        trig[:, h, :],
                        start=(c == 0 and h == 0),
                        stop=(c == NCH - 1 and h == 1))

            t1 = fin.tile([8, KP], f32)
            nc.vector.tensor_tensor(out=t1, in0=s_ps, in1=w8_t, op=Alu.mult)
            t2 = fin.tile([8, KP], f32)
            red = fin.tile([8, 1], f32)
            nc.scalar.activation(out=t2, in_=t1,
                                 func=mybir.ActivationFunctionType.Square,
                                 bias=zz_t[0:8, 0:1], accum_out=red)
            nc.sync.dma_start(out=out.ap(), in_=red)


_kernel_cache = {}


def kernel(q, r, cell, batch):
    global _last_results
    in_maps, meta = _host_prep(q, r, cell, batch)
    key = (meta["KP"], meta["NCH"], meta["CB"])
    if key not in _kernel_cache:
        _kernel_cache[key] = _build_kernel(*key)
    nc = _kernel_cache[key]

    trace = os.environ.get("EWALD_TRACE", "0") == "1"
    res = run_bass_kernel_spmd(nc, in_maps, core_ids=list(range(N_CORES)),
                               trace=trace)
    _last_results = res

    pot = np.zeros(16, np.float64)
    for m in range(N_CORES):
        red = res.results[m]["out"][:, 0].astype(np.float64)
        for bl in range(2):
            gb = 2 * m + bl
            pot[gb] = (red[4 * bl: 4 * bl + 4].sum() / meta["vol"][gb]
                       - 2.0 * meta["self_term"][gb])
    return (pot * NORM_FACTOR).astype(np.float32)


# revision 12
# speedup vs baseline: 1.0791x; 1.0791x over previous
"""Ewald summation kernel for Trainium2 (8 NeuronCores, SPMD).

Strategy
--------
Host (numpy, O(B*K + N) work):
  * Build the 21^3 integer reciprocal lattice, mirror the reference's fp32
    weight computation, and keep only k-points with nonzero weight
    (~460 of 9261 per batch -- exact: zero-weight points contribute 0).
  * Shard by batch: B=16 batches over 8 cores, 2 per core.  Each core
    computes complete structure factors for its two batches, so no
    collective is needed; per-core output is 8 partial scalars.
  * Work in "turns": phase/2pi = nvec . f where f = inv(cell) r are
    fractional coordinates.  f is wrapped to [-1/2, 1/2) and split into
    a 12-bit head + tail (both exact in float32r, as are the integer
    nvec), so ONE single-pass float32r matmul with contraction 6 gives
    the phase to fp32 accuracy.  The wrap shifts phases by multiples of
    1/2 turn per k-point, which only flips the sign of (sin, cos)
    together -- invariant under |S|^2.

Device (per core, Bass/Tile), per 128-atom chunk (NCH chunks, uniform):
  PE  : ph[128, KP] = f6^T @ nv6            (float32r, one matmul)
  DVE : d_sin = ph - round(ph)              (custom ROUND_REDUCE op)
        d_cos = wrap(d_sin + 1/4)           (ADD_RANGE_WRAP custom op)
  ACT : trig[128, 2KP] = Sin(2pi * [d_sin | d_cos])   -> float32r
  PE  : S[8, KP] += q8^T @ trig_half        (float32r matmuls; psum rows
        are (batch_local, half, channel))
tail: t2 = (S*w8)*S, reduce_sum -> [8,1] -> DMA out.

Host combines: pot[b] = sum(rows of b)/vol[b] - 2*self_term[b], * NORM.
"""

import os
import numpy as np

import concourse.bass as bass
import concourse.tile as tile
from concourse import bacc, mybir
from concourse.bass_utils import run_bass_kernel_spmd

# --- problem constants (from the reference model) -------------------------
N_MAX = 10
DL = 2.0
SIGMA = 1.0
NORM_FACTOR = 90.0474
TWOPI = 2.0 * np.pi
K_SQ_MAX = (TWOPI / DL) ** 2
SIGMA_SQ_HALF = SIGMA ** 2 / 2.0

N_CORES = 8
MAGIC = float(1.5 * 2 ** 23)  # fp32 round-to-nearest-integer magic constant

_last_results = None  # BassKernelResults of the most recent run (for test.py)


def _register_round_ops():
    """Custom DVE ops:
    ROUND_REDUCE_ANT:       out = in0 - ((in0 + s0) - s0) = in0 - round(in0)
    SHIFT_ROUND_REDUCE_ANT: out = y - ((y + s0) - s0), y = in0 + s1
    (s0 = fp32 magic rounding constant)."""
    import concourse.dve_ops as dve_ops
    from concourse.dve_spec import Spec, Src0, C0, C1, C2, PageIdx, lower
    from concourse.dve_uop import DveOpSpec

    def reg(name, spec, subdim=False):
        for op in dve_ops.OPS:
            if op.name == name:
                return op
        row = dve_ops._CUSTOM_DVE_ROW_BASE + len(dve_ops.OPS)
        assert row < 0x20
        dve_ops._SUB_OPCODE_FOR_NAME[name] = row
        shas = {}
        for ver in ("v3", "v4"):
            sp = DveOpSpec(name=name, opcode=row, uops=lower(spec, ver=ver),
                           rd1_en=False)
            shas[ver] = sp.sha(ver)
        op = dve_ops.DveOp(name, spec, subdim=subdim, uops_sha=shas)
        dve_ops.OPS.append(op)
        dve_ops.CUSTOM_DVE_SPECS[name] = spec
        return op

    r1 = reg("ROUND_REDUCE_ANT", Spec(
        body=Src0 - ((Src0 + C0) - C0),
        reference=lambda in0, in1, s0, s1, imm2: (
            in0 - ((in0 + s0) - s0)).astype(np.float32)))
    r2 = reg("SHIFT_ROUND_REDUCE_ANT", Spec(
        body=(Src0 + C1) - (((Src0 + C1) + C0) - C0),
        reference=lambda in0, in1, s0, s1, imm2: (
            (in0 + s1) - (((in0 + s1) + s0) - s0)).astype(np.float32)))

    def _pref(in0, in1, s0, s1, imm2):
        out = np.empty_like(in0)
        for pg in range(in0.shape[1]):
            y = in0[:, pg, :] + (s0 + s1 * pg)
            out[:, pg, :] = y - ((y + imm2) - imm2)
        return out.astype(np.float32)

    _y = Src0 + PageIdx(C0, C1)
    r3 = reg3 = None
    r3 = reg("PAGED_ROUND_REDUCE_ANT", Spec(
        body=_y - ((_y + C2) - C2),
        reference=_pref), subdim=True)
    return r1, r2, r3


def _k_lattice():
    g = np.arange(-N_MAX, N_MAX + 1)
    nvec = np.stack(np.meshgrid(g, g, g, indexing="ij"), axis=-1).reshape(-1, 3)
    nonzero = nvec != 0
    has_nz = nonzero.any(axis=1)
    first_nz = np.argmax(nonzero.astype(np.int32), axis=1)
    sign = nvec[np.arange(nvec.shape[0]), first_nz]
    hemi = (sign > 0) | ~has_nz
    factors = np.where(~has_nz, 1.0, 2.0).astype(np.float32)
    return nvec, hemi, factors


def _host_prep(q, r, cell, batch):
    """All O(B*K + N) prep.  Returns per-core input maps + combine info."""
    q = np.asarray(q, np.float32)
    r = np.asarray(r, np.float32)
    cell = np.asarray(cell, np.float32)
    batch = np.asarray(batch)
    B = cell.shape[0]
    assert B == 2 * N_CORES, f"batch-pair sharding assumes B=16, got {B}"

    nvec, hemi, factors = _k_lattice()

    # fp32 weight computation mirroring the reference
    inv32 = np.linalg.inv(cell).astype(np.float32)          # [B,3,3]
    G = (TWOPI * np.transpose(inv32, (0, 2, 1))).astype(np.float32)
    kvec = np.einsum("kj,bji->bki", nvec.astype(np.float32), G).astype(np.float32)
    k_sq = (kvec ** 2).sum(-1)
    valid = (k_sq > 0) & (k_sq <= np.float32(K_SQ_MAX)) & hemi[None, :]
    w = (np.exp(-np.float32(SIGMA_SQ_HALF) * k_sq) / (k_sq + 1e-12)
         * factors[None, :] * valid)

    inv64 = np.linalg.inv(cell.astype(np.float64))          # [B,3,3]

    sel_idx = [np.nonzero(w[b])[0] for b in range(B)]
    kmax = max(len(i) for i in sel_idx)
    KP = min(512, ((kmax + 15) // 16) * 16)
    assert kmax <= 512, f"valid k-points {kmax} > 512 unsupported"

    nsel = np.zeros((B, KP, 3), np.float32)                 # integer nvec
    wsel = np.zeros((B, KP), np.float32)
    for b in range(B):
        idx = sel_idx[b]
        nsel[b, : len(idx)] = nvec[idx]
        wsel[b, : len(idx)] = w[b][idx]

    counts = np.bincount(batch, minlength=B)
    starts = np.concatenate([[0], np.cumsum(counts)[:-1]])
    CB = int(max((counts + 127) // 128))
    NCH = 2 * CB

    # fractional coords wrapped to [-1/2, 1/2), split 12-bit head + tail
    f_all = np.einsum("bji,nj->nbi", inv64, r.astype(np.float64))  # [N,B,3]
    vol = np.linalg.det(cell.astype(np.float64))
    q_sq = q.astype(np.float64) ** 2
    self_term = np.array(
        [q_sq[batch == b].sum() for b in range(B)]) / (SIGMA * TWOPI ** 1.5)

    in_maps = []
    for m in range(N_CORES):
        f6 = np.zeros((6, NCH * 128), np.float32)
        q8 = np.zeros((128, NCH * 2 * 8), np.float32)
        nv6 = np.zeros((6, 2 * KP), np.float32)
        w8 = np.zeros((8, KP), np.float32)
        for bl in range(2):
            gb = 2 * m + bl
            nv6[0:3, bl * KP:(bl + 1) * KP] = nsel[gb].T
            nv6[3:6, bl * KP:(bl + 1) * KP] = nsel[gb].T
            for h in range(2):
                for c in range(2):
                    w8[4 * bl + 2 * h + c] = wsel[gb]
            nb = int(counts[gb])
            s0 = int(starts[gb])
            for j in range(CB):
                ch = bl * CB + j
                lo = j * 128
                n_here = max(0, min(128, nb - lo))
                if n_here == 0:
                    continue
                rows = slice(s0 + lo, s0 + lo + n_here)
                fb = f_all[rows, gb, :]                     # [n,3] fp64
                fb = np.mod(fb, 1.0) - 0.5
                fh = np.trunc(fb * 4096.0) / 4096.0
                fl = (fb - fh).astype(np.float32)
                f6[0:3, ch * 128: ch * 128 + n_here] = fh.T.astype(np.float32)
                f6[3:6, ch * 128: ch * 128 + n_here] = fl.T
                for h in range(2):
                    base = (2 * ch + h) * 8
                    for c in range(2):
                        q8[:n_here, base + 4 * bl + 2 * h + c] = q[rows, c]
        fnv = np.concatenate([f6[:, 0:128], nv6[:, 0:KP],
                              f6[:, 128:], nv6[:, KP:]], axis=1)
        q8z = np.concatenate([q8, np.zeros((128, 1), np.float32)], axis=1)
        in_maps.append({"fnv": fnv, "q8z": q8z,
                        "w8": np.sqrt(w8).astype(np.float32)})

    meta = dict(KP=KP, CB=CB, NCH=NCH, vol=vol, self_term=self_term)
    return in_maps, meta


def _build_kernel(KP, NCH, CB):
    rop, rop2, rop3 = _register_round_ops()

    orig_barrier = bass.Bass.all_engine_barrier
    orig_memset = bass.BassGpSimd.memset
    bass.Bass.all_engine_barrier = lambda self, **kw: None
    bass.BassGpSimd.memset = lambda self, ap, constant: None
    try:
        nc = bacc.Bacc("TRN2", target_bir_lowering=False, debug=False,
                       num_devices=N_CORES, detect_race_conditions=False,
                       enable_partition_id=False, monotonic_sem_count=0)
    finally:
        bass.Bass.all_engine_barrier = orig_barrier
        bass.BassGpSimd.memset = orig_memset

    f32 = mybir.dt.float32
    f32r = mybir.dt.float32r
    fnv = nc.dram_tensor("fnv", [6, NCH * 128 + 2 * KP], f32r,
                         kind="ExternalInput")
    q8z = nc.dram_tensor("q8z", [128, NCH * 2 * 8 + 1], f32r,
                         kind="ExternalInput")
    w8 = nc.dram_tensor("w8", [8, KP], f32, kind="ExternalInput")
    out = nc.dram_tensor("out", [8, 1], f32, kind="ExternalOutput")

    # slim exit: drain + one sem-only barrier + sem clear (the NEFF-level
    # postamble barrier provides the final sync for re-execution)
    def _slim_drain_and_barrier(self, tick_clock, wait_clock):
        from concourse.tile import ScopedClock
        drain_inst = self.nc.sync.drain()
        wait_clock.add_sem_waits(
            drain_inst.ins, ScopedClock({None: tick_clock.global_clock}))
        self.nc.all_engine_barrier(sem_only=True)
        popped = self.nc._tile_sem_poison_stack.pop()
        assert popped is self._sem_poison
        self.nc.clear_and_free_semaphores(
            list(self.sems.allocated().values()))

    Sin = mybir.ActivationFunctionType.Sin
    Alu = mybir.AluOpType

    orig_dab = tile.TileContext._drain_and_barrier
    tile.TileContext._drain_and_barrier = _slim_drain_and_barrier
    try:
        _build_body(nc, rop3, KP, NCH, CB, fnv, q8z, w8, out, Sin, Alu)
    finally:
        tile.TileContext._drain_and_barrier = orig_dab
    nc.compile()
    return nc


def _build_body(nc, rop3, KP, NCH, CB, fnv, q8z, w8, out, Sin, Alu):
    f32 = mybir.dt.float32
    f32r = mybir.dt.float32r
    with tile.TileContext(nc) as tc:
        with tc.tile_pool(name="consts", bufs=1) as consts, \
             tc.tile_pool(name="work", bufs=3) as work, \
             tc.tile_pool(name="fin", bufs=1) as fin, \
             tc.tile_pool(name="php", bufs=5, space="PSUM") as php, \
             tc.tile_pool(name="pss", bufs=1, space="PSUM") as pss:

            fnv_t = consts.tile([6, NCH * 128 + 2 * KP], f32r)
            # chunk-0 lhsT + batch-0 nvec first so the first matmul can start
            nc.sync.dma_start(out=fnv_t[:, 0:128 + KP],
                              in_=fnv.ap()[:, 0:128 + KP])
            nc.sync.dma_start(out=fnv_t[:, 128 + KP:],
                              in_=fnv.ap()[:, 128 + KP:])
            q8z_t = consts.tile([128, NCH * 2 * 8 + 1], f32r)
            w8_t = consts.tile([8, KP], f32)
            nc.sync.dma_start(out=q8z_t, in_=q8z.ap())
            nc.sync.dma_start(out=w8_t, in_=w8.ap())
            zz_t = q8z_t.bitcast(f32)[:, NCH * 2 * 8:]

            def f6_sl(c):
                if c == 0:
                    return fnv_t[:, 0:128]
                return fnv_t[:, KP + c * 128:KP + (c + 1) * 128]

            def nv6_sl(bl):
                if bl == 0:
                    return fnv_t[:, 128:128 + KP]
                return fnv_t[:, 128 + KP + (NCH - 1) * 128:
                             128 + KP + (NCH - 1) * 128 + KP]

            # PE warmup: dummy matmul with no input deps so the PE fetches
            # its instruction block while the first DMAs are in flight.
            warm = work.tile([1, 16], f32, tag="warm")
            nc.vector.memset(warm, 0.0)
            wps = pss.tile([16, 16], f32, tag="warmps")
            nc.tensor.matmul(wps, warm[0:1, :], warm[0:1, :],
                             start=True, stop=True)
            # ACT warmup: dummy Sin so the activation-table load (which
            # inherits the first user's waits) happens during the DMAs.
            warm2 = work.tile([1, 16], f32, tag="warm2")
            nc.scalar.activation(out=warm2, in_=warm, func=Sin,
                                 bias=0.0, scale=1.0)

            s_ps = pss.tile([8, KP], f32)

            for c in range(NCH):
                bl = c // CB
                ph = php.tile([128, KP], f32, tag="ph")
                nc.tensor.matmul(ph, f6_sl(c), nv6_sl(bl),
                                 start=True, stop=True)
                d2 = work.tile([128, 2, KP], f32, tag="d2")
                ph2 = bass.AP(tensor=ph.tensor, offset=ph.offset,
                              ap=[ph.ap[0], [0, 2], ph.ap[1]])
                # one paged op: page 0 -> sin d, page 1 -> +1/4 turn (cos)
                nc.vector._custom_dve(rop3, out=d2, in0=ph2,
                                      s0=0.0, s1=0.25, imm2=MAGIC)
                trig = work.tile([128, 2, KP], f32r, tag="trig")
                nc.scalar.activation(out=trig, in_=d2, func=Sin,
                                     bias=zz_t[:, 0:1], scale=float(TWOPI))
                for h in range(2):
                    nc.tensor.matmul(
                        s_ps[:, 0:KP],
                        q8z_t[:, (2 * c + h) * 8:(2 * c + h + 1) * 8],
                        trig[:, h, :],
                        start=(c == 0 and h == 0),
                        stop=(c == NCH - 1 and h == 1))

            t1 = fin.tile([8, KP], f32)
            nc.vector.tensor_tensor(out=t1, in0=s_ps, in1=w8_t, op=Alu.mult)
            t2 = fin.tile([8, KP], f32)
            red = fin.tile([8, 1], f32)
            nc.scalar.activation(out=t2, in_=t1,
                                 func=mybir.ActivationFunctionType.Square,
                                 bias=zz_t[0:8, 0:1], accum_out=red)
            nc.sync.dma_start(out=out.ap(), in_=red)


_kernel_cache = {}


def kernel(q, r, cell, batch):
    global _last_results
    in_maps, meta = _host_prep(q, r, cell, batch)
    key = (meta["KP"], meta["NCH"], meta["CB"])
    if key not in _kernel_cache:
        _kernel_cache[key] = _build_kernel(*key)
    nc = _kernel_cache[key]

    trace = os.environ.get("EWALD_TRACE", "0") == "1"
    res = run_bass_kernel_spmd(nc, in_maps, core_ids=list(range(N_CORES)),
                               trace=trace)
    _last_results = res

    pot = np.zeros(16, np.float64)
    for m in range(N_CORES):
        red = res.results[m]["out"][:, 0].astype(np.float64)
        for bl in range(2):
            gb = 2 * m + bl
            pot[gb] = (red[4 * bl: 4 * bl + 4].sum() / meta["vol"][gb]
                       - 2.0 * meta["self_term"][gb])
    return (pot * NORM_FACTOR).astype(np.float32)


# revision 16
# speedup vs baseline: 1.0878x; 1.0081x over previous
"""Ewald summation kernel for Trainium2 (8 NeuronCores, SPMD).

Strategy
--------
Host (numpy, O(B*K + N) work):
  * Build the 21^3 integer reciprocal lattice, mirror the reference's fp32
    weight computation, and keep only k-points with nonzero weight
    (~460 of 9261 per batch -- exact: zero-weight points contribute 0).
  * Shard by batch: B=16 batches over 8 cores, 2 per core.  Each core
    computes complete structure factors for its two batches, so no
    collective is needed; per-core output is 8 partial scalars.
  * Work in "turns": phase/2pi = nvec . f where f = inv(cell) r are
    fractional coordinates.  f is wrapped to [-1/2, 1/2) and split into
    a 12-bit head + tail (both exact in float32r, as are the integer
    nvec), so ONE single-pass float32r matmul with contraction 6 gives
    the phase to fp32 accuracy.  The wrap shifts phases by multiples of
    1/2 turn per k-point, which only flips the sign of (sin, cos)
    together -- invariant under |S|^2.

Device (per core, Bass/Tile), per 128-atom chunk (NCH chunks, uniform):
  PE  : ph[128, KP] = f6^T @ nv6            (float32r, one matmul)
  DVE : d_sin = ph - round(ph)              (custom ROUND_REDUCE op)
        d_cos = wrap(d_sin + 1/4)           (ADD_RANGE_WRAP custom op)
  ACT : trig[128, 2KP] = Sin(2pi * [d_sin | d_cos])   -> float32r
  PE  : S[8, KP] += q8^T @ trig_half        (float32r matmuls; psum rows
        are (batch_local, half, channel))
tail: t2 = (S*w8)*S, reduce_sum -> [8,1] -> DMA out.

Host combines: pot[b] = sum(rows of b)/vol[b] - 2*self_term[b], * NORM.
"""

import os
import numpy as np

import concourse.bass as bass
import concourse.tile as tile
from concourse import bacc, mybir
from concourse.bass_utils import run_bass_kernel_spmd

# --- problem constants (from the reference model) -------------------------
N_MAX = 10
DL = 2.0
SIGMA = 1.0
NORM_FACTOR = 90.0474
TWOPI = 2.0 * np.pi
K_SQ_MAX = (TWOPI / DL) ** 2
SIGMA_SQ_HALF = SIGMA ** 2 / 2.0

N_CORES = 8
MAGIC = float(1.5 * 2 ** 23)  # fp32 round-to-nearest-integer magic constant

_last_results = None  # BassKernelResults of the most recent run (for test.py)


def _register_round_ops():
    """Custom DVE ops:
    ROUND_REDUCE_ANT:       out = in0 - ((in0 + s0) - s0) = in0 - round(in0)
    SHIFT_ROUND_REDUCE_ANT: out = y - ((y + s0) - s0), y = in0 + s1
    (s0 = fp32 magic rounding constant)."""
    import concourse.dve_ops as dve_ops
    from concourse.dve_spec import Spec, Src0, C0, C1, C2, PageIdx, lower
    from concourse.dve_uop import DveOpSpec

    def reg(name, spec, subdim=False):
        for op in dve_ops.OPS:
            if op.name == name:
                return op
        row = dve_ops._CUSTOM_DVE_ROW_BASE + len(dve_ops.OPS)
        assert row < 0x20
        dve_ops._SUB_OPCODE_FOR_NAME[name] = row
        shas = {}
        for ver in ("v3", "v4"):
            sp = DveOpSpec(name=name, opcode=row, uops=lower(spec, ver=ver),
                           rd1_en=False)
            shas[ver] = sp.sha(ver)
        op = dve_ops.DveOp(name, spec, subdim=subdim, uops_sha=shas)
        dve_ops.OPS.append(op)
        dve_ops.CUSTOM_DVE_SPECS[name] = spec
        return op

    r1 = reg("ROUND_REDUCE_ANT", Spec(
        body=Src0 - ((Src0 + C0) - C0),
        reference=lambda in0, in1, s0, s1, imm2: (
            in0 - ((in0 + s0) - s0)).astype(np.float32)))
    r2 = reg("SHIFT_ROUND_REDUCE_ANT", Spec(
        body=(Src0 + C1) - (((Src0 + C1) + C0) - C0),
        reference=lambda in0, in1, s0, s1, imm2: (
            (in0 + s1) - (((in0 + s1) + s0) - s0)).astype(np.float32)))

    def _pref(in0, in1, s0, s1, imm2):
        out = np.empty_like(in0)
        for pg in range(in0.shape[1]):
            y = in0[:, pg, :] + (s0 + s1 * pg)
            out[:, pg, :] = y - ((y + imm2) - imm2)
        return out.astype(np.float32)

    _y = Src0 + PageIdx(C0, C1)
    r3 = reg3 = None
    r3 = reg("PAGED_ROUND_REDUCE_ANT", Spec(
        body=_y - ((_y + C2) - C2),
        reference=_pref), subdim=True)
    return r1, r2, r3


def _k_lattice():
    g = np.arange(-N_MAX, N_MAX + 1)
    nvec = np.stack(np.meshgrid(g, g, g, indexing="ij"), axis=-1).reshape(-1, 3)
    nonzero = nvec != 0
    has_nz = nonzero.any(axis=1)
    first_nz = np.argmax(nonzero.astype(np.int32), axis=1)
    sign = nvec[np.arange(nvec.shape[0]), first_nz]
    hemi = (sign > 0) | ~has_nz
    factors = np.where(~has_nz, 1.0, 2.0).astype(np.float32)
    return nvec, hemi, factors


def _host_prep(q, r, cell, batch):
    """All O(B*K + N) prep.  Returns per-core input maps + combine info."""
    q = np.asarray(q, np.float32)
    r = np.asarray(r, np.float32)
    cell = np.asarray(cell, np.float32)
    batch = np.asarray(batch)
    B = cell.shape[0]
    assert B == 2 * N_CORES, f"batch-pair sharding assumes B=16, got {B}"

    nvec, hemi, factors = _k_lattice()

    # fp32 weight computation mirroring the reference
    inv32 = np.linalg.inv(cell).astype(np.float32)          # [B,3,3]
    G = (TWOPI * np.transpose(inv32, (0, 2, 1))).astype(np.float32)
    kvec = np.einsum("kj,bji->bki", nvec.astype(np.float32), G).astype(np.float32)
    k_sq = (kvec ** 2).sum(-1)
    valid = (k_sq > 0) & (k_sq <= np.float32(K_SQ_MAX)) & hemi[None, :]
    w = (np.exp(-np.float32(SIGMA_SQ_HALF) * k_sq) / (k_sq + 1e-12)
         * factors[None, :] * valid)

    inv64 = np.linalg.inv(cell.astype(np.float64))          # [B,3,3]

    sel_idx = [np.nonzero(w[b])[0] for b in range(B)]
    kmax = max(len(i) for i in sel_idx)
    KP = min(512, ((kmax + 15) // 16) * 16)
    assert kmax <= 512, f"valid k-points {kmax} > 512 unsupported"

    nsel = np.zeros((B, KP, 3), np.float32)                 # integer nvec
    wsel = np.zeros((B, KP), np.float32)
    for b in range(B):
        idx = sel_idx[b]
        nsel[b, : len(idx)] = nvec[idx]
        wsel[b, : len(idx)] = w[b][idx]

    counts = np.bincount(batch, minlength=B)
    starts = np.concatenate([[0], np.cumsum(counts)[:-1]])
    # 32-atom groups, normalized across cores for SPMD
    G0 = int(max((counts[0::2] + 31) // 32))
    G1 = int(max((counts[1::2] + 31) // 32))
    NG = G0 + G1
    NCH = (NG + 3) // 4          # chunks of 4 group-slots
    # chunk segments: contiguous same-batch partition runs, fixed per slot
    slot_bl = [0 if g < G0 else 1 for g in range(NCH * 4)]
    segs = []
    for ch in range(NCH):
        runs = []
        for sl in range(4):
            bl = slot_bl[ch * 4 + sl]
            if runs and runs[-1][2] == bl:
                runs[-1] = (runs[-1][0], runs[-1][1] + 32, bl)
            else:
                runs.append((sl * 32, 32, bl))
        segs.append(tuple(runs))
    segs = tuple(segs)

    # fractional coords wrapped to [-1/2, 1/2), split 12-bit head + tail
    f_all = np.einsum("bji,nj->nbi", inv64, r.astype(np.float64))  # [N,B,3]
    vol = np.linalg.det(cell.astype(np.float64))
    q_sq = q.astype(np.float64) ** 2
    self_term = np.array(
        [q_sq[batch == b].sum() for b in range(B)]) / (SIGMA * TWOPI ** 1.5)

    in_maps = []
    for m in range(N_CORES):
        f12 = np.zeros((12, NCH * 128), np.float32)
        q8 = np.zeros((128, NCH * 2 * 8), np.float32)
        nv12 = np.zeros((12, KP), np.float32)
        w8 = np.zeros((8, KP), np.float32)
        for bl in range(2):
            gb = 2 * m + bl
            nv12[6 * bl + 0:6 * bl + 3] = nsel[gb].T
            nv12[6 * bl + 3:6 * bl + 6] = nsel[gb].T
            for h in range(2):
                for c in range(2):
                    w8[4 * bl + 2 * h + c] = wsel[gb]
            nb = int(counts[gb])
            s0 = int(starts[gb])
            gbase = 0 if bl == 0 else G0
            ng = (nb + 31) // 32
            for j in range(ng):
                slot = gbase + j
                ch, sl = divmod(slot, 4)
                lo = j * 32
                n_here = min(32, nb - lo)
                rows = slice(s0 + lo, s0 + lo + n_here)
                col0 = ch * 128 + sl * 32
                fb = f_all[rows, gb, :]                     # [n,3] fp64
                fb = np.mod(fb, 1.0) - 0.5
                fh = np.trunc(fb * 4096.0) / 4096.0
                fl = (fb - fh).astype(np.float32)
                r0 = 6 * bl
                f12[r0 + 0:r0 + 3, col0:col0 + n_here] = fh.T.astype(np.float32)
                f12[r0 + 3:r0 + 6, col0:col0 + n_here] = fl.T
                p0 = sl * 32
                for h in range(2):
                    base = (2 * ch + h) * 8
                    for c in range(2):
                        q8[p0:p0 + n_here,
                           base + 4 * bl + 2 * h + c] = q[rows, c]
        fnv = np.concatenate([f12[:, 0:128], nv12,
                              f12[:, 128:]], axis=1)
        q8z = np.concatenate([q8, np.zeros((128, 1), np.float32)], axis=1)
        in_maps.append({"fnv": fnv, "q8z": q8z,
                        "w8": np.sqrt(w8).astype(np.float32)})

    meta = dict(KP=KP, NCH=NCH, segs=segs, vol=vol, self_term=self_term)
    return in_maps, meta


def _build_kernel(KP, NCH, segs):
    rop, rop2, rop3 = _register_round_ops()

    orig_barrier = bass.Bass.all_engine_barrier
    orig_memset = bass.BassGpSimd.memset
    bass.Bass.all_engine_barrier = lambda self, **kw: None
    bass.BassGpSimd.memset = lambda self, ap, constant: None
    try:
        nc = bacc.Bacc("TRN2", target_bir_lowering=False, debug=False,
                       num_devices=N_CORES, detect_race_conditions=False,
                       enable_partition_id=False, monotonic_sem_count=0)
    finally:
        bass.Bass.all_engine_barrier = orig_barrier
        bass.BassGpSimd.memset = orig_memset

    f32 = mybir.dt.float32
    f32r = mybir.dt.float32r
    fnv = nc.dram_tensor("fnv", [12, NCH * 128 + KP], f32r,
                         kind="ExternalInput")
    q8z = nc.dram_tensor("q8z", [128, NCH * 2 * 8 + 1], f32r,
                         kind="ExternalInput")
    w8 = nc.dram_tensor("w8", [8, KP], f32, kind="ExternalInput")
    out = nc.dram_tensor("out", [8, 1], f32, kind="ExternalOutput")

    # slim exit: drain + one sem-only barrier + sem clear (the NEFF-level
    # postamble barrier provides the final sync for re-execution)
    def _slim_drain_and_barrier(self, tick_clock, wait_clock):
        from concourse.tile import ScopedClock
        drain_inst = self.nc.sync.drain()
        wait_clock.add_sem_waits(
            drain_inst.ins, ScopedClock({None: tick_clock.global_clock}))
        self.nc.all_engine_barrier(sem_only=True)
        popped = self.nc._tile_sem_poison_stack.pop()
        assert popped is self._sem_poison
        self.nc.clear_and_free_semaphores(
            list(self.sems.allocated().values()))

    Sin = mybir.ActivationFunctionType.Sin
    Alu = mybir.AluOpType

    orig_dab = tile.TileContext._drain_and_barrier
    tile.TileContext._drain_and_barrier = _slim_drain_and_barrier
    try:
        _build_body(nc, rop3, KP, NCH, segs, fnv, q8z, w8, out, Sin, Alu)
    finally:
        tile.TileContext._drain_and_barrier = orig_dab
    nc.compile()
    return nc


def _build_body(nc, rop3, KP, NCH, segs, fnv, q8z, w8, out, Sin, Alu):
    f32 = mybir.dt.float32
    f32r = mybir.dt.float32r
    with tile.TileContext(nc) as tc:
        with tc.tile_pool(name="consts", bufs=1) as consts, \
             tc.tile_pool(name="work", bufs=3) as work, \
             tc.tile_pool(name="fin", bufs=1) as fin, \
             tc.tile_pool(name="php", bufs=5, space="PSUM") as php, \
             tc.tile_pool(name="pss", bufs=1, space="PSUM") as pss:

            fnv_t = consts.tile([12, NCH * 128 + KP], f32r)
            # chunk-0 lhsT + batch-0 nvec first so the first matmul can start
            nc.sync.dma_start(out=fnv_t[:, 0:128 + KP],
                              in_=fnv.ap()[:, 0:128 + KP])
            nc.sync.dma_start(out=fnv_t[:, 128 + KP:],
                              in_=fnv.ap()[:, 128 + KP:])
            q8z_t = consts.tile([128, NCH * 2 * 8 + 1], f32r)
            w8_t = consts.tile([8, KP], f32)
            nc.sync.dma_start(out=q8z_t, in_=q8z.ap())
            nc.sync.dma_start(out=w8_t, in_=w8.ap())
            zz_t = q8z_t.bitcast(f32)[:, NCH * 2 * 8:]

            def f12_sl(c):
                if c == 0:
                    return fnv_t[:, 0:128]
                return fnv_t[:, KP + c * 128:KP + (c + 1) * 128]

            nv12_sl = fnv_t[:, 128:128 + KP]

            # PE warmup: dummy matmul with no input deps so the PE fetches
            # its instruction block while the first DMAs are in flight.
            warm = work.tile([1, 16], f32, tag="warm")
            nc.vector.memset(warm, 0.0)
            wps = pss.tile([16, 16], f32, tag="warmps")
            nc.tensor.matmul(wps, warm[0:1, :], warm[0:1, :],
                             start=True, stop=True)
            # ACT warmup: dummy Sin so the activation-table load (which
            # inherits the first user's waits) happens during the DMAs.
            warm2 = work.tile([1, 16], f32, tag="warm2")
            nc.scalar.activation(out=warm2, in_=warm, func=Sin,
                                 bias=warm[0:1, 0:1], scale=1.0)

            s_ps = pss.tile([8, KP], f32)

            for c in range(NCH):
                ph = php.tile([128, KP], f32, tag="ph")
                nc.tensor.matmul(ph, f12_sl(c), nv12_sl,
                                 start=True, stop=True)
                d2 = work.tile([128, 2, KP], f32, tag="d2")
                ph2 = bass.AP(tensor=ph.tensor, offset=ph.offset,
                              ap=[ph.ap[0], [0, 2], ph.ap[1]])
                # one paged op: page 0 -> sin d, page 1 -> +1/4 turn (cos)
                nc.vector._custom_dve(rop3, out=d2, in0=ph2,
                                      s0=0.0, s1=0.25, imm2=MAGIC)
                trig = work.tile([128, 2, KP], f32r, tag="trig")
                nc.scalar.activation(out=trig, in_=d2, func=Sin,
                                     bias=zz_t[:, 0:1], scale=float(TWOPI))
                for h in range(2):
                    nc.tensor.matmul(
                        s_ps[:, 0:KP],
                        q8z_t[:, (2 * c + h) * 8:(2 * c + h + 1) * 8],
                        trig[:, h, :],
                        start=(c == 0 and h == 0),
                        stop=(c == NCH - 1 and h == 1))

            t1 = fin.tile([8, KP], f32)
            nc.vector.tensor_tensor(out=t1, in0=s_ps, in1=w8_t, op=Alu.mult)
            t2 = fin.tile([8, KP], f32)
            red = fin.tile([8, 1], f32)
            nc.scalar.activation(out=t2, in_=t1,
                                 func=mybir.ActivationFunctionType.Square,
                                 bias=zz_t[0:8, 0:1], accum_out=red)
            nc.sync.dma_start(out=out.ap(), in_=red)


_kernel_cache = {}


def kernel(q, r, cell, batch):
    global _last_results
    in_maps, meta = _host_prep(q, r, cell, batch)
    key = (meta["KP"], meta["NCH"], meta["segs"])
    if key not in _kernel_cache:
        _kernel_cache[key] = _build_kernel(*key)
    nc = _kernel_cache[key]

    trace = os.environ.get("EWALD_TRACE", "0") == "1"
    res = run_bass_kernel_spmd(nc, in_maps, core_ids=list(range(N_CORES)),
                               trace=trace)
    _last_results = res

    pot = np.zeros(16, np.float64)
    for m in range(N_CORES):
        red = res.results[m]["out"][:, 0].astype(np.float64)
        for bl in range(2):
            gb = 2 * m + bl
            pot[gb] = (red[4 * bl: 4 * bl + 4].sum() / meta["vol"][gb]
                       - 2.0 * meta["self_term"][gb])
    return (pot * NORM_FACTOR).astype(np.float32)


# revision 18
# speedup vs baseline: 1.1531x; 1.0600x over previous
"""Ewald summation kernel for Trainium2 (8 NeuronCores, SPMD).

Strategy
--------
Host (numpy, O(B*K + N) work):
  * Build the 21^3 integer reciprocal lattice, mirror the reference's fp32
    weight computation, and keep only k-points with nonzero weight
    (~460 of 9261 per batch -- exact: zero-weight points contribute 0).
  * Shard by batch: B=16 batches over 8 cores, 2 per core.  Each core
    computes complete structure factors for its two batches, so no
    collective is needed; per-core output is 8 partial scalars.
  * Work in "turns": phase/2pi = nvec . f where f = inv(cell) r are
    fractional coordinates.  f is wrapped to [-1/2, 1/2) and split into
    a 12-bit head + tail (both exact in float32r, as are the integer
    nvec), so ONE single-pass float32r matmul with contraction 6 gives
    the phase to fp32 accuracy.  The wrap shifts phases by multiples of
    1/2 turn per k-point, which only flips the sign of (sin, cos)
    together -- invariant under |S|^2.

Device (per core, Bass/Tile), per 128-atom chunk (NCH chunks, uniform):
  PE  : ph[128, KP] = f6^T @ nv6            (float32r, one matmul)
  DVE : d_sin = ph - round(ph)              (custom ROUND_REDUCE op)
        d_cos = wrap(d_sin + 1/4)           (ADD_RANGE_WRAP custom op)
  ACT : trig[128, 2KP] = Sin(2pi * [d_sin | d_cos])   -> float32r
  PE  : S[8, KP] += q8^T @ trig_half        (float32r matmuls; psum rows
        are (batch_local, half, channel))
tail: t2 = (S*w8)*S, reduce_sum -> [8,1] -> DMA out.

Host combines: pot[b] = sum(rows of b)/vol[b] - 2*self_term[b], * NORM.
"""

import os
import numpy as np

import concourse.bass as bass
import concourse.tile as tile
from concourse import bacc, mybir
from concourse.bass_utils import run_bass_kernel_spmd

# --- problem constants (from the reference model) -------------------------
N_MAX = 10
DL = 2.0
SIGMA = 1.0
NORM_FACTOR = 90.0474
TWOPI = 2.0 * np.pi
K_SQ_MAX = (TWOPI / DL) ** 2
SIGMA_SQ_HALF = SIGMA ** 2 / 2.0

N_CORES = 8
MAGIC = float(1.5 * 2 ** 23)  # fp32 round-to-nearest-integer magic constant

_last_results = None  # BassKernelResults of the most recent run (for test.py)


def _register_round_ops():
    """Custom DVE ops:
    ROUND_REDUCE_ANT:       out = in0 - ((in0 + s0) - s0) = in0 - round(in0)
    SHIFT_ROUND_REDUCE_ANT: out = y - ((y + s0) - s0), y = in0 + s1
    (s0 = fp32 magic rounding constant)."""
    import concourse.dve_ops as dve_ops
    from concourse.dve_spec import Spec, Src0, C0, C1, C2, PageIdx, lower
    from concourse.dve_uop import DveOpSpec

    def reg(name, spec, subdim=False):
        for op in dve_ops.OPS:
            if op.name == name:
                return op
        row = dve_ops._CUSTOM_DVE_ROW_BASE + len(dve_ops.OPS)
        assert row < 0x20
        dve_ops._SUB_OPCODE_FOR_NAME[name] = row
        shas = {}
        for ver in ("v3", "v4"):
            sp = DveOpSpec(name=name, opcode=row, uops=lower(spec, ver=ver),
                           rd1_en=False)
            shas[ver] = sp.sha(ver)
        op = dve_ops.DveOp(name, spec, subdim=subdim, uops_sha=shas)
        dve_ops.OPS.append(op)
        dve_ops.CUSTOM_DVE_SPECS[name] = spec
        return op

    r1 = reg("ROUND_REDUCE_ANT", Spec(
        body=Src0 - ((Src0 + C0) - C0),
        reference=lambda in0, in1, s0, s1, imm2: (
            in0 - ((in0 + s0) - s0)).astype(np.float32)))
    r2 = reg("SHIFT_ROUND_REDUCE_ANT", Spec(
        body=(Src0 + C1) - (((Src0 + C1) + C0) - C0),
        reference=lambda in0, in1, s0, s1, imm2: (
            (in0 + s1) - (((in0 + s1) + s0) - s0)).astype(np.float32)))

    def _pref(in0, in1, s0, s1, imm2):
        out = np.empty_like(in0)
        for pg in range(in0.shape[1]):
            y = in0[:, pg, :] + (s0 + s1 * pg)
            out[:, pg, :] = y - ((y + imm2) - imm2)
        return out.astype(np.float32)

    _y = Src0 + PageIdx(C0, C1)
    r3 = reg3 = None
    r3 = reg("PAGED_ROUND_REDUCE_ANT", Spec(
        body=_y - ((_y + C2) - C2),
        reference=_pref), subdim=True)
    return r1, r2, r3


def _k_lattice():
    g = np.arange(-N_MAX, N_MAX + 1)
    nvec = np.stack(np.meshgrid(g, g, g, indexing="ij"), axis=-1).reshape(-1, 3)
    nonzero = nvec != 0
    has_nz = nonzero.any(axis=1)
    first_nz = np.argmax(nonzero.astype(np.int32), axis=1)
    sign = nvec[np.arange(nvec.shape[0]), first_nz]
    hemi = (sign > 0) | ~has_nz
    factors = np.where(~has_nz, 1.0, 2.0).astype(np.float32)
    return nvec, hemi, factors


def _host_prep(q, r, cell, batch):
    """All O(B*K + N) prep.  Returns per-core input maps + combine info."""
    q = np.asarray(q, np.float32)
    r = np.asarray(r, np.float32)
    cell = np.asarray(cell, np.float32)
    batch = np.asarray(batch)
    B = cell.shape[0]
    assert B == 2 * N_CORES, f"batch-pair sharding assumes B=16, got {B}"

    nvec, hemi, factors = _k_lattice()

    # fp32 weight computation mirroring the reference
    inv32 = np.linalg.inv(cell).astype(np.float32)          # [B,3,3]
    G = (TWOPI * np.transpose(inv32, (0, 2, 1))).astype(np.float32)
    kvec = np.einsum("kj,bji->bki", nvec.astype(np.float32), G).astype(np.float32)
    k_sq = (kvec ** 2).sum(-1)
    valid = (k_sq > 0) & (k_sq <= np.float32(K_SQ_MAX)) & hemi[None, :]
    w = (np.exp(-np.float32(SIGMA_SQ_HALF) * k_sq) / (k_sq + 1e-12)
         * factors[None, :] * valid)

    inv64 = np.linalg.inv(cell.astype(np.float64))          # [B,3,3]

    sel_idx = [np.nonzero(w[b])[0] for b in range(B)]
    kmax = max(len(i) for i in sel_idx)
    KP = min(512, ((kmax + 15) // 16) * 16)
    assert kmax <= 512, f"valid k-points {kmax} > 512 unsupported"

    nsel = np.zeros((B, KP, 3), np.float32)                 # integer nvec
    wsel = np.zeros((B, KP), np.float32)
    for b in range(B):
        idx = sel_idx[b]
        nsel[b, : len(idx)] = nvec[idx]
        wsel[b, : len(idx)] = w[b][idx]

    counts = np.bincount(batch, minlength=B)
    starts = np.concatenate([[0], np.cumsum(counts)[:-1]])
    # 32-atom groups, normalized across cores for SPMD
    G0 = int(max((counts[0::2] + 31) // 32))
    G1 = int(max((counts[1::2] + 31) // 32))
    NG = G0 + G1
    NCH = (NG + 3) // 4          # chunks of 4 group-slots
    # chunk segments: contiguous same-batch partition runs, fixed per slot
    slot_bl = [0 if g < G0 else 1 for g in range(NCH * 4)]
    segs = []
    for ch in range(NCH):
        runs = []
        for sl in range(4):
            bl = slot_bl[ch * 4 + sl]
            if runs and runs[-1][2] == bl:
                runs[-1] = (runs[-1][0], runs[-1][1] + 32, bl)
            else:
                runs.append((sl * 32, 32, bl))
        segs.append(tuple(runs))
    segs = tuple(segs)

    # fractional coords wrapped to [-1/2, 1/2), split 12-bit head + tail
    f_all = np.einsum("bji,nj->nbi", inv64, r.astype(np.float64))  # [N,B,3]
    vol = np.linalg.det(cell.astype(np.float64))
    q_sq = q.astype(np.float64) ** 2
    self_term = np.array(
        [q_sq[batch == b].sum() for b in range(B)]) / (SIGMA * TWOPI ** 1.5)

    in_maps = []
    for m in range(N_CORES):
        f12 = np.zeros((12, NCH * 128), np.float32)
        q8 = np.zeros((128, NCH * 2 * 8), np.float32)
        nv12 = np.zeros((12, KP), np.float32)
        w8 = np.zeros((8, KP), np.float32)
        for bl in range(2):
            gb = 2 * m + bl
            nv12[6 * bl + 0:6 * bl + 3] = nsel[gb].T
            nv12[6 * bl + 3:6 * bl + 6] = nsel[gb].T
            for h in range(2):
                for c in range(2):
                    w8[4 * bl + 2 * h + c] = wsel[gb]
            nb = int(counts[gb])
            s0 = int(starts[gb])
            gbase = 0 if bl == 0 else G0
            ng = (nb + 31) // 32
            for j in range(ng):
                slot = gbase + j
                ch, sl = divmod(slot, 4)
                lo = j * 32
                n_here = min(32, nb - lo)
                rows = slice(s0 + lo, s0 + lo + n_here)
                col0 = ch * 128 + sl * 32
                fb = f_all[rows, gb, :]                     # [n,3] fp64
                fb = np.mod(fb, 1.0) - 0.5
                fh = np.trunc(fb * 4096.0) / 4096.0
                fl = (fb - fh).astype(np.float32)
                r0 = 6 * bl
                f12[r0 + 0:r0 + 3, col0:col0 + n_here] = fh.T.astype(np.float32)
                f12[r0 + 3:r0 + 6, col0:col0 + n_here] = fl.T
                p0 = sl * 32
                for h in range(2):
                    base = (2 * ch + h) * 8
                    for c in range(2):
                        q8[p0:p0 + n_here,
                           base + 4 * bl + 2 * h + c] = q[rows, c]
        fnv = np.concatenate([f12[:, 0:128], nv12,
                              f12[:, 128:]], axis=1)
        q8z = np.concatenate([q8, np.zeros((128, 1), np.float32)], axis=1)
        in_maps.append({"fnv": fnv, "q8z": q8z,
                        "w8": np.sqrt(w8).astype(np.float32)})

    meta = dict(KP=KP, NCH=NCH, segs=segs, vol=vol, self_term=self_term)
    return in_maps, meta


def _build_kernel(KP, NCH, segs):
    rop, rop2, rop3 = _register_round_ops()

    orig_barrier = bass.Bass.all_engine_barrier
    orig_memset = bass.BassGpSimd.memset
    bass.Bass.all_engine_barrier = lambda self, **kw: None
    bass.BassGpSimd.memset = lambda self, ap, constant: None
    try:
        nc = bacc.Bacc("TRN2", target_bir_lowering=False, debug=False,
                       num_devices=N_CORES, detect_race_conditions=False,
                       enable_partition_id=False, monotonic_sem_count=0)
    finally:
        bass.Bass.all_engine_barrier = orig_barrier
        bass.BassGpSimd.memset = orig_memset

    f32 = mybir.dt.float32
    f32r = mybir.dt.float32r
    fnv = nc.dram_tensor("fnv", [12, NCH * 128 + KP], f32r,
                         kind="ExternalInput")
    q8z = nc.dram_tensor("q8z", [128, NCH * 2 * 8 + 1], f32r,
                         kind="ExternalInput")
    w8 = nc.dram_tensor("w8", [8, KP], f32, kind="ExternalInput")
    out = nc.dram_tensor("out", [8, 1], f32, kind="ExternalOutput")

    # slim exit: drain + one sem-only barrier + sem clear (the NEFF-level
    # postamble barrier provides the final sync for re-execution)
    def _slim_drain_and_barrier(self, tick_clock, wait_clock):
        from concourse.tile import ScopedClock
        drain_inst = self.nc.sync.drain()
        wait_clock.add_sem_waits(
            drain_inst.ins, ScopedClock({None: tick_clock.global_clock}))
        done = self.nc.alloc_semaphore("tile_exit_done")
        drain_inst.then_inc(done, 1)
        self.nc.gpsimd.wait_ge(done, 1)
        popped = self.nc._tile_sem_poison_stack.pop()
        assert popped is self._sem_poison
        self.nc.clear_and_free_semaphores(
            list(self.sems.allocated().values()) + [done])

    Sin = mybir.ActivationFunctionType.Sin
    Alu = mybir.AluOpType

    orig_dab = tile.TileContext._drain_and_barrier
    tile.TileContext._drain_and_barrier = _slim_drain_and_barrier
    try:
        _build_body(nc, rop3, KP, NCH, segs, fnv, q8z, w8, out, Sin, Alu)
    finally:
        tile.TileContext._drain_and_barrier = orig_dab
    nc.compile()
    return nc


def _build_body(nc, rop3, KP, NCH, segs, fnv, q8z, w8, out, Sin, Alu):
    f32 = mybir.dt.float32
    f32r = mybir.dt.float32r
    with tile.TileContext(nc) as tc:
        with tc.tile_pool(name="consts", bufs=1) as consts, \
             tc.tile_pool(name="work", bufs=3) as work, \
             tc.tile_pool(name="fin", bufs=1) as fin, \
             tc.tile_pool(name="php", bufs=5, space="PSUM") as php, \
             tc.tile_pool(name="pss", bufs=1, space="PSUM") as pss:

            fnv_t = consts.tile([12, NCH * 128 + KP], f32r)
            # chunk-0 lhsT + batch-0 nvec first so the first matmul can start
            nc.sync.dma_start(out=fnv_t[:, 0:128 + KP],
                              in_=fnv.ap()[:, 0:128 + KP])
            nc.sync.dma_start(out=fnv_t[:, 128 + KP:],
                              in_=fnv.ap()[:, 128 + KP:])
            q8z_t = consts.tile([128, NCH * 2 * 8 + 1], f32r)
            w8_t = consts.tile([8, KP], f32)
            nc.sync.dma_start(out=q8z_t, in_=q8z.ap())
            nc.sync.dma_start(out=w8_t, in_=w8.ap())
            zz_t = q8z_t.bitcast(f32)[:, NCH * 2 * 8:]

            def f12_sl(c):
                if c == 0:
                    return fnv_t[:, 0:128]
                return fnv_t[:, KP + c * 128:KP + (c + 1) * 128]

            nv12_sl = fnv_t[:, 128:128 + KP]

            # PE warmup: dummy matmul with no input deps so the PE fetches
            # its instruction block while the first DMAs are in flight.
            warm = work.tile([1, 16], f32, tag="warm")
            nc.vector.memset(warm, 0.0)
            wps = pss.tile([16, 16], f32, tag="warmps")
            nc.tensor.matmul(wps, warm[0:1, :], warm[0:1, :],
                             start=True, stop=True)
            # ACT warmup: dummy Sin so the activation-table load (which
            # inherits the first user's waits) happens during the DMAs.
            warm2 = work.tile([1, 16], f32, tag="warm2")
            nc.scalar.activation(out=warm2, in_=warm, func=Sin,
                                 bias=warm[0:1, 0:1], scale=1.0)

            s_ps = pss.tile([8, KP], f32)

            for c in range(NCH):
                ph = php.tile([128, KP], f32, tag="ph")
                nc.tensor.matmul(ph, f12_sl(c), nv12_sl,
                                 start=True, stop=True)
                d2 = work.tile([128, 2, KP], f32, tag="d2")
                ph2 = bass.AP(tensor=ph.tensor, offset=ph.offset,
                              ap=[ph.ap[0], [0, 2], ph.ap[1]])
                # one paged op: page 0 -> sin d, page 1 -> +1/4 turn (cos)
                nc.vector._custom_dve(rop3, out=d2, in0=ph2,
                                      s0=0.0, s1=0.25, imm2=MAGIC)
                trig = work.tile([128, 2, KP], f32r, tag="trig")
                nc.scalar.activation(out=trig, in_=d2, func=Sin,
                                     bias=zz_t[:, 0:1], scale=float(TWOPI))
                for h in range(2):
                    nc.tensor.matmul(
                        s_ps[:, 0:KP],
                        q8z_t[:, (2 * c + h) * 8:(2 * c + h + 1) * 8],
                        trig[:, h, :],
                        start=(c == 0 and h == 0),
                        stop=(c == NCH - 1 and h == 1))

            t1 = fin.tile([8, KP], f32)
            nc.vector.tensor_tensor(out=t1, in0=s_ps, in1=w8_t, op=Alu.mult)
            t2 = fin.tile([8, KP], f32)
            red = fin.tile([8, 1], f32)
            nc.scalar.activation(out=t2, in_=t1,
                                 func=mybir.ActivationFunctionType.Square,
                                 bias=zz_t[0:8, 0:1], accum_out=red)
            nc.sync.dma_start(out=out.ap(), in_=red)


_kernel_cache = {}


def kernel(q, r, cell, batch):
    global _last_results
    in_maps, meta = _host_prep(q, r, cell, batch)
    key = (meta["KP"], meta["NCH"], meta["segs"])
    if key not in _kernel_cache:
        _kernel_cache[key] = _build_kernel(*key)
    nc = _kernel_cache[key]

    trace = os.environ.get("EWALD_TRACE", "0") == "1"
    res = run_bass_kernel_spmd(nc, in_maps, core_ids=list(range(N_CORES)),
                               trace=trace)
    _last_results = res

    pot = np.zeros(16, np.float64)
    for m in range(N_CORES):
        red = res.results[m]["out"][:, 0].astype(np.float64)
        for bl in range(2):
            gb = 2 * m + bl
            pot[gb] = (red[4 * bl: 4 * bl + 4].sum() / meta["vol"][gb]
                       - 2.0 * meta["self_term"][gb])
    return (pot * NORM_FACTOR).astype(np.float32)
